# revision 2
# baseline (speedup 1.0000x reference)
"""CTAttention (dilated window attention) Trainium2 kernel, v2.

Self-contained: hardcodes shapes from the problem spec.
  N=500000 tokens, C=256, H=8 heads (hd=32), window K=24, dilation D=4.
  Block = K*D = 96 tokens; attention is block-diagonal over dilated windows.

v2 design vs baseline:
  - Host does the dilation reshuffle (win-order permute), pads, casts x to
    f16 and TRANSPOSES it per core -> device loads X^T [C, tok] directly.
    Kills all PE transposes of X and the psum->sbuf copies after them.
  - One input DMA + one mask DMA per supertile (4 blocks); two output DMAs
    per supertile. HWDGE fixed cost (625ns/DMA) drops ~4x.
  - S^T psum tiles are f16 (1 bank per block, 8 heads) -> one Exp per block.
  - proj bias folded on host (y += beff after gather).
  - psum->sbuf copies spread across DVE / ACT / Pool.
"""

import numpy as np

K = 24
D = 4
C = 256
H = 8
HD = 32
NTOK = 500000
BS = 8
BLOCK = K * D          # 96
NCORES = 8
NB = 652               # blocks per core
NBLKP = NCORES * NB    # 5216 padded blocks
TOK = NB * BLOCK       # 62592 tokens per core
NPAD = NBLKP * BLOCK   # 500736
SCALE = HD ** -0.5
SUP = 4                # blocks per supertile
NSUP = NB // SUP       # 163
SUPB = SUP * BLOCK     # 384


def build_nc(nb):
    """Build the Bass program for `nb` blocks per core (nb % SUP == 0)."""
    import concourse.bacc as bacc
    import concourse.bass as bass
    import concourse.tile as tile
    from concourse import mybir

    f32 = mybir.dt.float32
    f16 = mybir.dt.float16
    AF = mybir.ActivationFunctionType
    OP = mybir.AluOpType

    nsup = nb // SUP
    tok = nb * BLOCK

    nc = bacc.Bacc("TRN2", target_bir_lowering=False, debug=False,
                   num_devices=NCORES)

    xT = nc.declare_dram_parameter("xT", [C, tok], f16, isOutput=False)
    mk_d = nc.declare_dram_parameter("mk", [nsup, BLOCK, SUP * BLOCK], f16,
                                     isOutput=False)
    wqkv_d = nc.declare_dram_parameter("wqkvT", [C, 3 * C], f16,
                                       isOutput=False)
    bqk_d = nc.declare_dram_parameter("bqk", [128, 4], f32, isOutput=False)
    wp_d = nc.declare_dram_parameter("wpT", [C, C], f16, isOutput=False)
    id96_d = nc.declare_dram_parameter("id96", [BLOCK, BLOCK], f16,
                                       isOutput=False)
    y = nc.declare_dram_parameter("y", [tok, C], f32, isOutput=True)

    xTr = xT.rearrange("(a p) t -> p a t", p=128)
    yr = y.rearrange("(b g p) c -> b p g c", p=BLOCK, g=SUP)

    with tile.TileContext(nc) as tc:
        with (
            tc.tile_pool(name="const", bufs=1) as const,
            tc.tile_pool(name="xt", bufs=4) as xt_p,
            tc.tile_pool(name="qk", bufs=3) as qk_p,
            tc.tile_pool(name="mkp", bufs=3) as mk_p,
            tc.tile_pool(name="pt", bufs=5) as pt_p,
            tc.tile_pool(name="vv", bufs=3) as vv_p,
            tc.tile_pool(name="og", bufs=5) as og_p,
            tc.tile_pool(name="rc", bufs=5) as rc_p,
            tc.tile_pool(name="ot", bufs=3) as ot_p,
            tc.tile_pool(name="yo", bufs=3) as yo_p,
            tc.tile_pool(name="ps1", bufs=2, space="PSUM") as ps1,
            tc.tile_pool(name="pse", bufs=4, space="PSUM") as pse,
            tc.tile_pool(name="psy", bufs=2, space="PSUM") as psy,
        ):
            # ---- constants ----
            wq = const.tile([128, 2, 3 * C], f16)
            nc.sync.dma_start(out=wq[:],
                              in_=wqkv_d.rearrange("(a p) f -> p a f", p=128))
            wp = const.tile([128, 2, C], f16)
            nc.sync.dma_start(out=wp[:],
                              in_=wp_d.rearrange("(a p) f -> p a f", p=128))
            bqk = const.tile([128, 4], f32)
            nc.sync.dma_start(out=bqk[:], in_=bqk_d[:, :])
            id96 = const.tile([BLOCK, BLOCK], f16)
            nc.sync.dma_start(out=id96[:], in_=id96_d[:, :])

            def s_pair(it, qk, mk, gp):
                """S^T + exp for a PAIR of blocks. Each psum bank receives
                matmuls from a single PE-array strip only (they serialize),
                matching the baseline-safe pattern."""
                pt = pt_p.tile([BLOCK, 2, 4, 2, BLOCK], f16)
                for h4 in range(4):
                    rows = slice(32 * h4, 32 * h4 + 32)
                    pss = pse.tile([BLOCK, 2, 2, BLOCK], f32, tag="pse")
                    for hh in range(2):
                        for gg in range(2):
                            gc = slice((2 * gp + gg) * BLOCK,
                                       (2 * gp + gg + 1) * BLOCK)
                            nc.tensor.matmul(
                                pss[:, hh, gg, :],
                                lhsT=qk[rows, 2 + hh, gc],
                                rhs=qk[rows, hh, gc],
                                start=True, stop=True,
                                tile_position=(32 * h4, 0))
                    nc.scalar.activation(
                        out=pt[:, :, h4, :, :], in_=pss[:],
                        func=AF.Exp, scale=1.0)
                return pt

            def mask_gg(mk, gp, gg, pt):
                # split across DVE (heads 0-3) and Pool (heads 4-7): halves
                # DVE load; each PV head matmul only waits for its own half.
                g = 2 * gp + gg
                nc.vector.tensor_tensor(
                    out=pt[:, :, 0:2, gg, :], in0=pt[:, :, 0:2, gg, :],
                    in1=mk[:, g, None, None, :].to_broadcast(
                        (BLOCK, 2, 2, BLOCK)),
                    op=OP.mult)
                nc.gpsimd.tensor_tensor(
                    out=pt[:, :, 2:4, gg, :], in0=pt[:, :, 2:4, gg, :],
                    in1=mk[:, g, None, None, :].to_broadcast(
                        (BLOCK, 2, 2, BLOCK)),
                    op=OP.mult)

            def pv_block(g, pt, vv):
                """P @ V' + normalize for one block -> og tile."""
                gg = g % 2
                ops = ps1.tile([BLOCK, H, HD + 1], f32, tag="ps1")
                for h in range(H):
                    nc.tensor.matmul(
                        ops[:, h, :],
                        lhsT=pt[:, h // 4, h % 4, gg, :],
                        rhs=vv[:, gg, h, :],
                        start=True, stop=True)
                rc = rc_p.tile([BLOCK, H], f32)
                nc.vector.reciprocal(out=rc[:], in_=ops[:, :, HD])
                og = og_p.tile([BLOCK, H, HD], f16)
                nc.vector.tensor_tensor(
                    out=og[:], in0=ops[:, :, 0:HD],
                    in1=rc[:, :, None].to_broadcast((BLOCK, H, HD)),
                    op=OP.mult)
                return og

            def ot_stage(ogs):
                """O^T transposes + psum->sbuf copies for both pairs."""
                ots = []
                for gp in range(2):
                    otp = psy.tile([128, 2, 2, BLOCK], f16, tag="psy")
                    for gg in range(2):
                        for cc in range(2):
                            nc.tensor.transpose(
                                otp[:, gg, cc, :],
                                ogs[2 * gp + gg][:, 4 * cc:4 * cc + 4, :],
                                id96[:])
                    ot = ot_p.tile([128, 2, 2, BLOCK], f16)
                    nc.vector.tensor_copy(out=ot[:], in_=otp[:])
                    ots.append(ot)
                return ots

            def proj_stage(it, ots):
                for gp in range(2):
                    yps = psy.tile([BLOCK, 2, C], f32, tag="psy")
                    for gg in range(2):
                        for cc in range(2):
                            nc.tensor.matmul(
                                yps[:, gg, :], lhsT=ots[gp][:, gg, cc, :],
                                rhs=wp[:, cc, :],
                                start=(cc == 0), stop=(cc == 1))
                    yo = yo_p.tile([BLOCK, 2, C], f32)
                    nc.scalar.activation(out=yo[:], in_=yps[:],
                                         func=AF.Copy)
                    nc.sync.dma_start(out=yr[it, :, 2 * gp:2 * gp + 2], in_=yo[:])

            def stage_a(it, memset_ones):
                """Loads + QK^T + V matmuls + psum->sbuf moves for it."""
                t0 = it * SUPB
                xt = xt_p.tile([128, 2, SUPB], f16)
                nc.sync.dma_start(out=xt[:], in_=xTr[:, :, t0:t0 + SUPB])
                mk = mk_p.tile([BLOCK, SUP, BLOCK], f16)
                nc.sync.dma_start(
                    out=mk[:],
                    in_=mk_d[it].rearrange("p (g j) -> p g j", g=SUP))

                qk = qk_p.tile([128, 4, SUPB], f16)
                for ft in range(4):
                    qps = psy.tile([128, SUPB], f32, tag="psy")
                    for cc in range(2):
                        nc.tensor.matmul(
                            qps[:],
                            lhsT=wq[:, cc, ft * 128:(ft + 1) * 128],
                            rhs=xt[:, cc, :],
                            start=(cc == 0), stop=(cc == 1))
                    nc.vector.tensor_scalar(
                        out=qk[:, ft, :], in0=qps[:],
                        scalar1=bqk[:, ft:ft + 1], scalar2=None,
                        op0=OP.add)

                vvs = []
                for gp in range(2):
                    vps = ps1.tile([BLOCK, 2, C], f32, tag="ps1")
                    for gg in range(2):
                        gc = slice((2 * gp + gg) * BLOCK,
                                   (2 * gp + gg + 1) * BLOCK)
                        for cc in range(2):
                            nc.tensor.matmul(
                                vps[:, gg, :],
                                lhsT=xt[:, cc, gc],
                                rhs=wq[:, cc, 2 * C:3 * C],
                                start=(cc == 0), stop=(cc == 1))
                    vv = vv_p.tile([BLOCK, 2, H, HD + 1], f16)
                    nc.scalar.activation(
                        out=vv[:, :, :, 0:HD],
                        in_=vps.rearrange("p g (h d) -> p g h d", h=H),
                        func=AF.Copy)
                    if memset_ones:
                        nc.gpsimd.memset(vv[:, :, :, HD:HD + 1], 1.0)
                    vvs.append(vv)
                return qk, mk, vvs

            # Software pipeline: A runs one supertile ahead, the output
            # stage one behind; S-blocks issue early so exp/mask latency is
            # hidden under A(it+1) + out(it-1) PE work before PV needs them.
            VV_BUFS = 3
            cur = stage_a(0, True)
            prev = None
            for it in range(nsup):
                qk, mk, vvs = cur
                ptA = s_pair(it, qk, mk, 0)
                if prev is not None:
                    ots = ot_stage(prev[1])
                cur = (stage_a(it + 1, it + 1 < VV_BUFS)
                       if it + 1 < nsup else None)
                ptB = s_pair(it, qk, mk, 1)
                mask_gg(mk, 0, 0, ptA)
                mask_gg(mk, 0, 1, ptA)
                og0 = pv_block(0, ptA, vvs[0])
                og1 = pv_block(1, ptA, vvs[0])
                if prev is not None:
                    proj_stage(prev[0], ots)
                mask_gg(mk, 1, 0, ptB)
                mask_gg(mk, 1, 1, ptB)
                og2 = pv_block(2, ptB, vvs[1])
                og3 = pv_block(3, ptB, vvs[1])
                prev = (it, [og0, og1, og2, og3])
            ots = ot_stage(prev[1])
            proj_stage(prev[0], ots)

    nc.compile()
    return nc


def host_prep(data, qkv_w, qkv_b, proj_w, proj_b, batch_idx, ncores=NCORES,
              nb=NB):
    """Shard + preprocess inputs. Returns in_maps list for run_bass_kernel_spmd."""
    nblkp = ncores * nb
    npad = nblkp * BLOCK
    tok = nb * BLOCK
    nsup = nb // SUP

    n = data.shape[0]
    data_pad = np.zeros((npad, C), np.float32)
    data_pad[:n] = data
    batch_pad = np.full((npad,), BS, np.int32)
    batch_pad[:n] = batch_idx

    # win-order permute of tokens within each block: (k, w) -> (w, k)
    dwin = (data_pad.reshape(nblkp, K, D, C).transpose(0, 2, 1, 3)
            .reshape(npad, C).astype(np.float16))

    # categories in window order: block -> [k, w] -> win-order (w, k)
    cats = batch_pad.reshape(nblkp, K, D).transpose(0, 2, 1)  # [blk, w, k]
    cats = cats + 16 * np.arange(D, dtype=np.int32)[None, :, None]
    catw = cats.reshape(nblkp, BLOCK)
    mask01 = (catw[:, :, None] == catw[:, None, :]).astype(np.float16)
    # -> [core, nsup, 96, SUP*96] with layout [p, g, j]
    mk = (mask01.reshape(ncores, nsup, SUP, BLOCK, BLOCK)
          .transpose(0, 1, 3, 2, 4)
          .reshape(ncores, nsup, BLOCK, SUP * BLOCK).copy())

    wqkvT = np.ascontiguousarray(qkv_w.T).astype(np.float32)
    wqkvT[:, :C] *= SCALE
    wqkvT = wqkvT.astype(np.float16)
    bqk_full = qkv_b[:2 * C].astype(np.float32).copy()
    bqk_full[:C] *= SCALE
    bqk = np.ascontiguousarray(bqk_full.reshape(4, 128).T)
    wpT = np.ascontiguousarray(proj_w.T).astype(np.float16)
    id96 = np.eye(BLOCK, dtype=np.float16)

    in_maps = []
    for c in range(ncores):
        xT = np.ascontiguousarray(dwin[c * tok:(c + 1) * tok].T)
        in_maps.append({
            "xT": xT, "mk": mk[c], "wqkvT": wqkvT, "bqk": bqk,
            "wpT": wpT, "id96": id96,
        })
    return in_maps


_NC_CACHE = {}


def kernel(data, qkv_w, qkv_b, proj_w, proj_b, batch_idx):
    from concourse.bass_utils import run_bass_kernel_spmd

    data = np.asarray(data, np.float32)
    qkv_w = np.asarray(qkv_w, np.float32)
    qkv_b = np.asarray(qkv_b, np.float32)
    proj_w = np.asarray(proj_w, np.float32)
    proj_b = np.asarray(proj_b, np.float32)
    batch_idx = np.asarray(batch_idx, np.int32)

    if "nc" not in _NC_CACHE:
        _NC_CACHE["nc"] = build_nc(NB)
    nc = _NC_CACHE["nc"]

    in_maps = host_prep(data, qkv_w, qkv_b, proj_w, proj_b, batch_idx)
    res = run_bass_kernel_spmd(nc, in_maps, list(range(NCORES)))
    out = np.concatenate([res.results[c]["y"] for c in range(NCORES)], axis=0)
    # reverse the win-order permute, drop padding, add folded proj bias
    out = (out.reshape(NBLKP, D, K, C).transpose(0, 2, 1, 3)
           .reshape(NPAD, C)[:NTOK])
    beff = (proj_b + qkv_b[2 * C:] @ proj_w.T).astype(np.float32)
    return np.ascontiguousarray(out + beff[None, :])


# revision 3
# speedup vs baseline: 1.1441x; 1.1441x over previous
"""CTAttention (dilated window attention) Trainium2 kernel, v2.

Self-contained: hardcodes shapes from the problem spec.
  N=500000 tokens, C=256, H=8 heads (hd=32), window K=24, dilation D=4.
  Block = K*D = 96 tokens; attention is block-diagonal over dilated windows.

v2 design vs baseline:
  - Host does the dilation reshuffle (win-order permute), pads, casts x to
    f16 and TRANSPOSES it per core -> device loads X^T [C, tok] directly.
    Kills all PE transposes of X and the psum->sbuf copies after them.
  - One input DMA + one mask DMA per supertile (4 blocks); two output DMAs
    per supertile. HWDGE fixed cost (625ns/DMA) drops ~4x.
  - S^T psum tiles are f16 (1 bank per block, 8 heads) -> one Exp per block.
  - proj bias folded on host (y += beff after gather).
  - psum->sbuf copies spread across DVE / ACT / Pool.
"""

import numpy as np

K = 24
D = 4
C = 256
H = 8
HD = 32
NTOK = 500000
BS = 8
BLOCK = K * D          # 96
NCORES = 8
NB = 652               # blocks per core
NBLKP = NCORES * NB    # 5216 padded blocks
TOK = NB * BLOCK       # 62592 tokens per core
NPAD = NBLKP * BLOCK   # 500736
SCALE = HD ** -0.5
SUP = 4                # blocks per supertile
NSUP = NB // SUP       # 163
SUPB = SUP * BLOCK     # 384


def build_nc(nb):
    """Build the Bass program for `nb` blocks per core (nb % SUP == 0)."""
    import concourse.bacc as bacc
    import concourse.bass as bass
    import concourse.tile as tile
    from concourse import mybir

    f32 = mybir.dt.float32
    f16 = mybir.dt.float16
    AF = mybir.ActivationFunctionType
    OP = mybir.AluOpType

    nsup = nb // SUP
    tok = nb * BLOCK

    nc = bacc.Bacc("TRN2", target_bir_lowering=False, debug=False,
                   num_devices=NCORES)

    xT = nc.declare_dram_parameter("xT", [C, tok], f16, isOutput=False)
    mk_d = nc.declare_dram_parameter("mk", [nsup, BLOCK, SUP * BLOCK], f16,
                                     isOutput=False)
    wqkv_d = nc.declare_dram_parameter("wqkvT", [C, 3 * C], f16,
                                       isOutput=False)
    bqk_d = nc.declare_dram_parameter("bqk", [128, 4], f32, isOutput=False)
    wp_d = nc.declare_dram_parameter("wpT", [C, C], f16, isOutput=False)
    id96_d = nc.declare_dram_parameter("id96", [BLOCK, BLOCK], f16,
                                       isOutput=False)
    y = nc.declare_dram_parameter("y", [tok, C], f32, isOutput=True)

    xTr = xT.rearrange("(a p) t -> p a t", p=128)
    yr = y.rearrange("(b g p) c -> b p g c", p=BLOCK, g=SUP)

    with tile.TileContext(nc) as tc:
        with (
            tc.tile_pool(name="const", bufs=1) as const,
            tc.tile_pool(name="xt", bufs=4) as xt_p,
            tc.tile_pool(name="qk", bufs=3) as qk_p,
            tc.tile_pool(name="mkp", bufs=3) as mk_p,
            tc.tile_pool(name="pt", bufs=5) as pt_p,
            tc.tile_pool(name="vv", bufs=3) as vv_p,
            tc.tile_pool(name="og", bufs=5) as og_p,
            tc.tile_pool(name="rc", bufs=5) as rc_p,
            tc.tile_pool(name="ot", bufs=3) as ot_p,
            tc.tile_pool(name="yo", bufs=3) as yo_p,
            tc.tile_pool(name="ps1", bufs=2, space="PSUM") as ps1,
            tc.tile_pool(name="pse", bufs=2, space="PSUM") as pse,
            tc.tile_pool(name="psy", bufs=2, space="PSUM") as psy,
        ):
            # ---- constants ----
            wq = const.tile([128, 2, 3 * C], f16)
            nc.sync.dma_start(out=wq[:],
                              in_=wqkv_d.rearrange("(a p) f -> p a f", p=128))
            wp = const.tile([128, 2, C], f16)
            nc.sync.dma_start(out=wp[:],
                              in_=wp_d.rearrange("(a p) f -> p a f", p=128))
            bqk = const.tile([128, 4], f32)
            nc.sync.dma_start(out=bqk[:], in_=bqk_d[:, :])
            id96 = const.tile([BLOCK, BLOCK], f16)
            nc.sync.dma_start(out=id96[:], in_=id96_d[:, :])

            def s_pair(it, qk, mk, gp):
                """S^T + exp for a PAIR of blocks. The psum tile spans two
                banks; each BANK receives matmuls from a single PE-array
                strip only (same-strip matmuls serialize -> bank-safe), and
                one Exp covers both banks."""
                pt = pt_p.tile([BLOCK, 2, 2, 2, 2, BLOCK], f16)
                for t in range(2):
                    # [96, 2, 512] f32: each s-slice is exactly one psum
                    # bank; all matmuls into a bank come from one PE strip.
                    pss = pse.tile([BLOCK, 2, 512], f32, tag="pse")
                    for s in range(2):
                        h4 = 2 * t + s
                        rows = slice(32 * h4, 32 * h4 + 32)
                        for hh in range(2):
                            for gg in range(2):
                                gc = slice((2 * gp + gg) * BLOCK,
                                           (2 * gp + gg + 1) * BLOCK)
                                o = hh * 192 + gg * 96
                                nc.tensor.matmul(
                                    pss[:, s, o:o + BLOCK],
                                    lhsT=qk[rows, 2 + hh, gc],
                                    rhs=qk[rows, hh, gc],
                                    start=True, stop=True,
                                    tile_position=(32 * h4, 0))
                    nc.scalar.activation(
                        out=pt[:, t, :, :, :, :],
                        in_=pss[:, :, 0:384].rearrange(
                            "p s (a b q) -> p s a b q", a=2, b=2),
                        func=AF.Exp, scale=1.0)
                return pt

            def mask_gg(mk, gp, gg, pt):
                # split across DVE (strips 0-1) and Pool (strips 2-3): halves
                # DVE load; each PV head matmul only waits for its own half.
                g = 2 * gp + gg
                nc.vector.tensor_tensor(
                    out=pt[:, 0, :, :, gg, :], in0=pt[:, 0, :, :, gg, :],
                    in1=mk[:, g, None, None, :].to_broadcast(
                        (BLOCK, 2, 2, BLOCK)),
                    op=OP.mult)
                nc.gpsimd.tensor_tensor(
                    out=pt[:, 1, :, :, gg, :], in0=pt[:, 1, :, :, gg, :],
                    in1=mk[:, g, None, None, :].to_broadcast(
                        (BLOCK, 2, 2, BLOCK)),
                    op=OP.mult)

            def pv_block(g, pt, vv):
                """P @ V' + normalize for one block -> og tile."""
                gg = g % 2
                ops = ps1.tile([BLOCK, H, HD + 1], f32, tag="ps1")
                for h in range(H):
                    nc.tensor.matmul(
                        ops[:, h, :],
                        lhsT=pt[:, (h % 4) // 2, (h % 4) % 2, h // 4, gg, :],
                        rhs=vv[:, gg, h, :],
                        start=True, stop=True)
                rc = rc_p.tile([BLOCK, H], f32)
                nc.vector.reciprocal(out=rc[:], in_=ops[:, :, HD])
                og = og_p.tile([BLOCK, H, HD], f16)
                nc.vector.tensor_tensor(
                    out=og[:], in0=ops[:, :, 0:HD],
                    in1=rc[:, :, None].to_broadcast((BLOCK, H, HD)),
                    op=OP.mult)
                return og

            def ot_stage(ogs):
                """O^T transposes + psum->sbuf copies for both pairs."""
                ots = []
                for gp in range(2):
                    otp = psy.tile([128, 2, 2, BLOCK], f16, tag="psy")
                    for gg in range(2):
                        for cc in range(2):
                            nc.tensor.transpose(
                                otp[:, gg, cc, :],
                                ogs[2 * gp + gg][:, 4 * cc:4 * cc + 4, :],
                                id96[:])
                    ot = ot_p.tile([128, 2, 2, BLOCK], f16)
                    nc.vector.tensor_copy(out=ot[:], in_=otp[:])
                    ots.append(ot)
                return ots

            def proj_stage(it, ots):
                for gp in range(2):
                    yps = psy.tile([BLOCK, 2, C], f32, tag="psy")
                    for gg in range(2):
                        for cc in range(2):
                            nc.tensor.matmul(
                                yps[:, gg, :], lhsT=ots[gp][:, gg, cc, :],
                                rhs=wp[:, cc, :],
                                start=(cc == 0), stop=(cc == 1))
                    yo = yo_p.tile([BLOCK, 2, C], f32)
                    nc.scalar.activation(out=yo[:], in_=yps[:],
                                         func=AF.Copy)
                    nc.sync.dma_start(out=yr[it, :, 2 * gp:2 * gp + 2], in_=yo[:])

            def stage_a(it, memset_ones):
                """Loads + QK^T + V matmuls + psum->sbuf moves for it."""
                t0 = it * SUPB
                xt = xt_p.tile([128, 2, SUPB], f16)
                nc.sync.dma_start(out=xt[:], in_=xTr[:, :, t0:t0 + SUPB])
                mk = mk_p.tile([BLOCK, SUP, BLOCK], f16)
                nc.sync.dma_start(
                    out=mk[:],
                    in_=mk_d[it].rearrange("p (g j) -> p g j", g=SUP))

                qk = qk_p.tile([128, 4, SUPB], f16)
                for ft in range(4):
                    qps = psy.tile([128, SUPB], f32, tag="psy")
                    for cc in range(2):
                        nc.tensor.matmul(
                            qps[:],
                            lhsT=wq[:, cc, ft * 128:(ft + 1) * 128],
                            rhs=xt[:, cc, :],
                            start=(cc == 0), stop=(cc == 1))
                    nc.vector.tensor_scalar(
                        out=qk[:, ft, :], in0=qps[:],
                        scalar1=bqk[:, ft:ft + 1], scalar2=None,
                        op0=OP.add)

                vvs = []
                for gp in range(2):
                    vps = ps1.tile([BLOCK, 2, C], f32, tag="ps1")
                    for gg in range(2):
                        gc = slice((2 * gp + gg) * BLOCK,
                                   (2 * gp + gg + 1) * BLOCK)
                        for cc in range(2):
                            nc.tensor.matmul(
                                vps[:, gg, :],
                                lhsT=xt[:, cc, gc],
                                rhs=wq[:, cc, 2 * C:3 * C],
                                start=(cc == 0), stop=(cc == 1))
                    vv = vv_p.tile([BLOCK, 2, H, HD + 1], f16)
                    nc.scalar.activation(
                        out=vv[:, :, :, 0:HD],
                        in_=vps.rearrange("p g (h d) -> p g h d", h=H),
                        func=AF.Copy)
                    if memset_ones:
                        nc.gpsimd.memset(vv[:, :, :, HD:HD + 1], 1.0)
                    vvs.append(vv)
                return qk, mk, vvs

            # Software pipeline: A runs one supertile ahead, the output
            # stage one behind; S-blocks issue early so exp/mask latency is
            # hidden under A(it+1) + out(it-1) PE work before PV needs them.
            VV_BUFS = 3
            cur = stage_a(0, True)
            prev = None
            for it in range(nsup):
                qk, mk, vvs = cur
                ptA = s_pair(it, qk, mk, 0)
                if prev is not None:
                    ots = ot_stage(prev[1])
                cur = (stage_a(it + 1, it + 1 < VV_BUFS)
                       if it + 1 < nsup else None)
                ptB = s_pair(it, qk, mk, 1)
                mask_gg(mk, 0, 0, ptA)
                mask_gg(mk, 0, 1, ptA)
                og0 = pv_block(0, ptA, vvs[0])
                og1 = pv_block(1, ptA, vvs[0])
                if prev is not None:
                    proj_stage(prev[0], ots)
                mask_gg(mk, 1, 0, ptB)
                mask_gg(mk, 1, 1, ptB)
                og2 = pv_block(2, ptB, vvs[1])
                og3 = pv_block(3, ptB, vvs[1])
                prev = (it, [og0, og1, og2, og3])
            ots = ot_stage(prev[1])
            proj_stage(prev[0], ots)

    nc.compile()
    return nc


def host_prep(data, qkv_w, qkv_b, proj_w, proj_b, batch_idx, ncores=NCORES,
              nb=NB):
    """Shard + preprocess inputs. Returns in_maps list for run_bass_kernel_spmd."""
    nblkp = ncores * nb
    npad = nblkp * BLOCK
    tok = nb * BLOCK
    nsup = nb // SUP

    n = data.shape[0]
    data_pad = np.zeros((npad, C), np.float32)
    data_pad[:n] = data
    batch_pad = np.full((npad,), BS, np.int32)
    batch_pad[:n] = batch_idx

    # win-order permute of tokens within each block: (k, w) -> (w, k)
    dwin = (data_pad.reshape(nblkp, K, D, C).transpose(0, 2, 1, 3)
            .reshape(npad, C).astype(np.float16))

    # categories in window order: block -> [k, w] -> win-order (w, k)
    cats = batch_pad.reshape(nblkp, K, D).transpose(0, 2, 1)  # [blk, w, k]
    cats = cats + 16 * np.arange(D, dtype=np.int32)[None, :, None]
    catw = cats.reshape(nblkp, BLOCK)
    mask01 = (catw[:, :, None] == catw[:, None, :]).astype(np.float16)
    # -> [core, nsup, 96, SUP*96] with layout [p, g, j]
    mk = (mask01.reshape(ncores, nsup, SUP, BLOCK, BLOCK)
          .transpose(0, 1, 3, 2, 4)
          .reshape(ncores, nsup, BLOCK, SUP * BLOCK).copy())

    wqkvT = np.ascontiguousarray(qkv_w.T).astype(np.float32)
    wqkvT[:, :C] *= SCALE
    wqkvT = wqkvT.astype(np.float16)
    bqk_full = qkv_b[:2 * C].astype(np.float32).copy()
    bqk_full[:C] *= SCALE
    bqk = np.ascontiguousarray(bqk_full.reshape(4, 128).T)
    wpT = np.ascontiguousarray(proj_w.T).astype(np.float16)
    id96 = np.eye(BLOCK, dtype=np.float16)

    in_maps = []
    for c in range(ncores):
        xT = np.ascontiguousarray(dwin[c * tok:(c + 1) * tok].T)
        in_maps.append({
            "xT": xT, "mk": mk[c], "wqkvT": wqkvT, "bqk": bqk,
            "wpT": wpT, "id96": id96,
        })
    return in_maps


_NC_CACHE = {}


def kernel(data, qkv_w, qkv_b, proj_w, proj_b, batch_idx):
    from concourse.bass_utils import run_bass_kernel_spmd

    data = np.asarray(data, np.float32)
    qkv_w = np.asarray(qkv_w, np.float32)
    qkv_b = np.asarray(qkv_b, np.float32)
    proj_w = np.asarray(proj_w, np.float32)
    proj_b = np.asarray(proj_b, np.float32)
    batch_idx = np.asarray(batch_idx, np.int32)

    if "nc" not in _NC_CACHE:
        _NC_CACHE["nc"] = build_nc(NB)
    nc = _NC_CACHE["nc"]

    in_maps = host_prep(data, qkv_w, qkv_b, proj_w, proj_b, batch_idx)
    res = run_bass_kernel_spmd(nc, in_maps, list(range(NCORES)))
    out = np.concatenate([res.results[c]["y"] for c in range(NCORES)], axis=0)
    # reverse the win-order permute, drop padding, add folded proj bias
    out = (out.reshape(NBLKP, D, K, C).transpose(0, 2, 1, 3)
           .reshape(NPAD, C)[:NTOK])
    beff = (proj_b + qkv_b[2 * C:] @ proj_w.T).astype(np.float32)
    return np.ascontiguousarray(out + beff[None, :])
